# revision 15
# baseline (speedup 1.0000x reference)
"""Trainium2 Bass kernel for the LRU (Linear Recurrent Unit) nn.Module.

Math
----
Reference computes, per timestep t (T=4096, H=2048, N=1024):
    Bu_t   = B_norm @ u_t                    (complex, B_norm = (B_re+iB_im)*gamma)
    h_t    = lambda * h_{t-1} + Bu_t         (diagonal complex recurrence)
    y_t    = Re(C @ h_t) + D * u_t

Device strategy (8 NeuronCores, tensor-parallel over d_hidden N):
Each core owns NSH = N/8 = 128 channels.  With lambda_n = r_n * exp(i*theta_n)
the rotating-frame substitution g_t = exp(-i*theta*t) * h_t turns the complex
recurrence into two *real* scans
    g_t = r * g_{t-1} + exp(-i*theta*t) * Bu_t
which map 1:1 onto the VectorE tensor_tensor_scan instruction.  Rotation
tables cos(theta_n*t), sin(theta_n*t) are precomputed on host in float64.

Per core:
  mm1  (TensorE):  Bu.T = BnT.T @ x.T        -> [NSH, T] (re,im) in PSUM
  rot-in (VectorE): w = exp(-i theta t) Bu   -> SBUF
  scan (VectorE):  g = scan(r, w)            (chunked, carried via `initial`)
  rot-out (VectorE): h = exp(+i theta t) g   -> f32r SBUF
  mm2  (TensorE):  y_part = h_re.T @ C_re.T - h_im.T @ C_im.T  -> [T, H]
Host gathers: y = sum_c y_part_c + D * u.

The emission is software-pipelined: mm2 of chunk c is emitted after mm1 of
chunk c+1 so the TensorE never waits on the VectorE scan chain (keeps the
PE HAM clock warm).
"""

import os

import numpy as np

T, H, N = 4096, 2048, 1024
NCORES = 8
NSH = N // NCORES  # 128 channels per core
TCH = 512          # time chunk (= max fp32 matmul moving free dim = 1 PSUM bank)
NCHUNK = T // TCH  # 8
KT = H // 128      # 16 contraction tiles in mm1
HCH = 512          # h chunk in mm2
NHC = H // HCH     # 4

_CACHE = {}

# last BassKernelResults (for test harness introspection)
last_results = None


def _build_program():
    import concourse.mybir as mybir
    from concourse import bacc
    from concourse.tile import TileContext

    F32 = mybir.dt.float32
    BF16 = mybir.dt.bfloat16
    MUL = mybir.AluOpType.mult
    ADD = mybir.AluOpType.add
    SUB = mybir.AluOpType.subtract

    nc = bacc.Bacc("TRN2", target_bir_lowering=False, debug=False,
                   num_devices=NCORES)

    xT = nc.dram_tensor("xT", [128, NCHUNK * KT * TCH], BF16,
                        kind="ExternalInput").ap()
    bn_re = nc.dram_tensor("bn_re", [128, KT * NSH], BF16,
                           kind="ExternalInput").ap()
    bn_im = nc.dram_tensor("bn_im", [128, KT * NSH], BF16,
                           kind="ExternalInput").ap()
    ct_re = nc.dram_tensor("ct_re", [NSH, H], BF16, kind="ExternalInput").ap()
    ct_in = nc.dram_tensor("ct_in", [NSH, H], BF16, kind="ExternalInput").ap()
    # merged per-chunk rotation table: [128, c, (cos|sin), TCH]
    csT = nc.dram_tensor("csT", [NSH, NCHUNK * 2 * TCH], BF16,
                         kind="ExternalInput").ap()
    rvec = nc.dram_tensor("rvec", [NSH, 1], F32, kind="ExternalInput").ap()
    ypart = nc.dram_tensor("ypart", [T, H], BF16, kind="ExternalOutput").ap()

    with TileContext(nc) as tc:
        with (
            tc.tile_pool(name="persist", bufs=1) as pp,
            tc.tile_pool(name="xin", bufs=2) as xp,
            tc.tile_pool(name="rot", bufs=2) as rp,
            tc.tile_pool(name="wbuf", bufs=3) as wp,
            tc.tile_pool(name="gbuf", bufs=3) as gp,
            tc.tile_pool(name="hbuf", bufs=3) as hp,
            tc.tile_pool(name="yout", bufs=3) as yp,
            tc.tile_pool(name="csn", bufs=3) as cp,
            tc.tile_pool(name="ps1", bufs=2, space="PSUM") as ps1,
            tc.tile_pool(name="ps2", bufs=4, space="PSUM") as ps2,
        ):
            # ---- persistent loads ----
            # Critical path: the first matmuls need bn slices + the first x
            # pieces.  Issue those in fine grains split across the two HWDGE
            # queues (sync + scalar) so the PE can start ~2us in.
            bre = pp.tile([128, KT * NSH], BF16, tag="bre")
            bim = pp.tile([128, KT * NSH], BF16, tag="bim")
            nc.scalar.dma_start(bre[:, 0:2 * NSH], bn_re[:, 0:2 * NSH])
            nc.scalar.dma_start(bim[:, 0:2 * NSH], bn_im[:, 0:2 * NSH])
            nc.scalar.dma_start(bre[:, 2 * NSH:], bn_re[:, 2 * NSH:])
            nc.scalar.dma_start(bim[:, 2 * NSH:], bn_im[:, 2 * NSH:])
            rv = pp.tile([128, 1], F32, tag="rv")
            nc.sync.dma_start(rv[:], rvec)
            ctr = pp.tile([128, H], BF16, tag="ctr")
            nc.gpsimd.dma_start(ctr[:], ct_re)
            cti = pp.tile([128, H], BF16, tag="cti")
            nc.gpsimd.dma_start(cti[:], ct_in)
            rbc = pp.tile([128, TCH], F32, tag="rbc")
            nc.vector.tensor_copy(rbc[:], rv[:, 0:1].broadcast_to([128, TCH]))

            prev_gre = prev_gim = None
            hist = []  # pending (chunk, hre, him) awaiting mm2

            def emit_front(c):
                """mm1 + rotations + scans for chunk c."""
                nonlocal prev_gre, prev_gim
                t0 = c * TCH
                # chunk 0 lands in eight fine pieces alternating between the
                # two HWDGE queues (PE can start on piece 0); later chunks
                # arrive as two halves on the sync queue, prefetched a full
                # chunk ahead.
                xt = xp.tile([128, KT * TCH], BF16, tag="xt")
                x0 = c * KT * TCH
                npc = 8 if c == 0 else 2
                PW = KT * TCH // npc
                for q in range(npc):
                    eng = nc.sync if (c > 0 or q % 2 == 0) else nc.scalar
                    eng.dma_start(
                        xt[:, q * PW:(q + 1) * PW],
                        xT[:, x0 + q * PW:x0 + (q + 1) * PW],
                    )
                pre = ps1.tile([128, TCH], F32, tag="pre")
                pim = ps1.tile([128, TCH], F32, tag="pim")
                for a in range(KT):
                    xsl = xt[:, a * TCH:(a + 1) * TCH]
                    nc.tensor.matmul(
                        pre[:], bre[:, a * NSH:(a + 1) * NSH], xsl,
                        start=(a == 0), stop=(a == KT - 1),
                    )
                    nc.tensor.matmul(
                        pim[:], bim[:, a * NSH:(a + 1) * NSH], xsl,
                        start=(a == 0), stop=(a == KT - 1),
                    )
                cs_t = cp.tile([128, 2 * TCH], BF16, tag="csl")
                nc.gpsimd.dma_start(
                    cs_t[:], csT[:, c * 2 * TCH:(c + 1) * 2 * TCH])
                csl = cs_t[:, 0:TCH]
                snl = cs_t[:, TCH:2 * TCH]
                # rotate into the r-frame: w = e^{-i theta t} * Bu
                t1 = rp.tile([128, TCH], F32, tag="t1")
                t2 = rp.tile([128, TCH], F32, tag="t2")
                wre = wp.tile([128, TCH], F32, tag="wre")
                wim = wp.tile([128, TCH], F32, tag="wim")
                nc.vector.tensor_tensor(t1[:], csl, pre[:], op=MUL)
                nc.vector.tensor_tensor(t2[:], snl, pim[:], op=MUL)
                nc.vector.tensor_tensor(wre[:], t1[:], t2[:], op=ADD)
                nc.vector.tensor_tensor(t1[:], csl, pim[:], op=MUL)
                nc.vector.tensor_tensor(t2[:], snl, pre[:], op=MUL)
                nc.vector.tensor_tensor(wim[:], t1[:], t2[:], op=SUB)
                # the two real scans
                gre = gp.tile([128, TCH], F32, tag="gre")
                gim = gp.tile([128, TCH], F32, tag="gim")
                init_re = 0.0 if c == 0 else prev_gre[:, TCH - 1:TCH]
                init_im = 0.0 if c == 0 else prev_gim[:, TCH - 1:TCH]
                nc.vector.tensor_tensor_scan(
                    gre[:], rbc[:], wre[:], init_re, MUL, ADD)
                nc.vector.tensor_tensor_scan(
                    gim[:], rbc[:], wim[:], init_im, MUL, ADD)
                prev_gre, prev_gim = gre, gim
                # rotate back: h = e^{+i theta t} * g
                hre = hp.tile([128, TCH], BF16, tag="hre")
                him = hp.tile([128, TCH], BF16, tag="him")
                nc.vector.tensor_tensor(t1[:], csl, gre[:], op=MUL)
                nc.vector.tensor_tensor(t2[:], snl, gim[:], op=MUL)
                nc.vector.tensor_tensor(hre[:], t1[:], t2[:], op=SUB)
                nc.vector.tensor_tensor(t1[:], csl, gim[:], op=MUL)
                nc.vector.tensor_tensor(t2[:], snl, gre[:], op=MUL)
                nc.vector.tensor_tensor(him[:], t1[:], t2[:], op=ADD)
                hist.append((c, hre, him))

            def emit_back():
                """mm2 + output for the oldest pending chunk."""
                c, hre, him = hist.pop(0)
                t0 = c * TCH
                last_c = c == NCHUNK - 1
                for tt in range(TCH // 128):
                    lre = hre[:, tt * 128:(tt + 1) * 128]
                    lim = him[:, tt * 128:(tt + 1) * 128]
                    last_tt = last_c and tt == TCH // 128 - 1
                    yo = yp.tile([128, H], BF16, tag="yo")
                    pos = []
                    for _ in range(NHC):
                        po = ps2.tile([128, HCH], F32, tag="po")
                        pos.append(po)
                    for hc in range(NHC):
                        nc.tensor.matmul(
                            pos[hc][:], lre, ctr[:, hc * HCH:(hc + 1) * HCH],
                            start=True, stop=False,
                        )
                    # close each bank then copy it out immediately (only the
                    # Scalar engine has PSUM access + spare cycles); ypart
                    # stores go on the sync queue so Scalar does copies only.
                    for hc in range(NHC):
                        nc.tensor.matmul(
                            pos[hc][:], lim, cti[:, hc * HCH:(hc + 1) * HCH],
                            start=False, stop=True,
                        )
                        nc.scalar.copy(yo[:, hc * HCH:(hc + 1) * HCH],
                                       pos[hc][:])
                        if last_tt:
                            nc.sync.dma_start(
                                ypart[t0 + tt * 128:t0 + (tt + 1) * 128,
                                      hc * HCH:(hc + 1) * HCH],
                                yo[:, hc * HCH:(hc + 1) * HCH])
                    if not last_tt:
                        nc.sync.dma_start(
                            ypart[t0 + tt * 128:t0 + (tt + 1) * 128, :],
                            yo[:])

            for c in range(NCHUNK):
                emit_front(c)
                if c >= 1:
                    emit_back()
            emit_back()

    nc.compile()
    return nc


def _arrange_bn(bn_slice):
    import ml_dtypes
    # bn_slice [NSH, H] (float64) -> [128, KT*NSH] with
    # out[p, a*NSH + n] = bn_slice[n, a*128 + p]
    bnT = bn_slice.T.astype(ml_dtypes.bfloat16)  # [H, NSH]
    return np.ascontiguousarray(
        bnT.reshape(KT, 128, NSH).transpose(1, 0, 2)).reshape(128, -1)


def _host_prep(inputs, nu, theta, gamma_log, B_re, B_im, C_re, C_im, D):
    """Float64 host-side precompute; returns per-core input maps."""
    import ml_dtypes
    BF = ml_dtypes.bfloat16
    x = np.asarray(inputs, dtype=np.float32)
    th64 = np.exp(np.asarray(theta).astype(np.float64))
    r64 = np.exp(-np.exp(np.asarray(nu).astype(np.float64)))
    gamma = np.exp(np.asarray(gamma_log).astype(np.float64))
    Bn_re = np.asarray(B_re).astype(np.float64) * gamma[:, None]
    Bn_im = np.asarray(B_im).astype(np.float64) * gamma[:, None]
    t_idx = np.arange(T, dtype=np.float64)
    phase = th64[:, None] * t_idx[None, :]
    cos_all = np.cos(phase).astype(BF)  # [N, T]
    sin_all = np.sin(phase).astype(BF)
    # merged per-chunk layout: cs_all[n, c, 0|1, t] = cos|sin(theta_n*(c*TCH+t))
    cs_all = np.stack(
        [cos_all.reshape(N, NCHUNK, TCH), sin_all.reshape(N, NCHUNK, TCH)],
        axis=2).reshape(N, NCHUNK * 2 * TCH)
    # pre-arrange x into the per-chunk SBUF layout:
    # xTa[p, c, a, t] = x[c*TCH + t, a*128 + p]
    xTa = np.ascontiguousarray(
        x.reshape(NCHUNK, TCH, KT, 128).transpose(3, 0, 2, 1).astype(BF)
    ).reshape(128, -1)
    C_re = np.asarray(C_re, dtype=np.float32).astype(BF)
    C_im = np.asarray(C_im, dtype=np.float32).astype(BF)

    in_maps = []
    for c in range(NCORES):
        sl = slice(c * NSH, (c + 1) * NSH)
        in_maps.append({
            "xT": xTa,
            "bn_re": _arrange_bn(Bn_re[sl]),
            "bn_im": _arrange_bn(Bn_im[sl]),
            "ct_re": np.ascontiguousarray(C_re[:, sl].T),
            "ct_in": np.ascontiguousarray(-C_im[:, sl].T),
            "csT": np.ascontiguousarray(cs_all[sl]),
            "rvec": np.ascontiguousarray(r64[sl].astype(np.float32)[:, None]),
        })
    return in_maps


def kernel(inputs, nu, theta, gamma_log, B_re, B_im, C_re, C_im, D):
    global last_results
    from concourse.bass_utils import run_bass_kernel_spmd

    if "nc" not in _CACHE:
        _CACHE["nc"] = _build_program()
    nc = _CACHE["nc"]

    in_maps = _host_prep(
        inputs, nu, theta, gamma_log, B_re, B_im, C_re, C_im, D)

    trace = os.environ.get("LRU_TRACE") == "1"
    res = run_bass_kernel_spmd(
        nc, in_maps, core_ids=list(range(NCORES)), trace=trace)
    last_results = res

    y64 = np.zeros((T, H), np.float64)
    for r in res.results:
        y64 += r["ypart"].astype(np.float64)
    y64 += (np.asarray(D).astype(np.float64)[None, :]
            * np.asarray(inputs).astype(np.float64))
    return y64.astype(np.float32)



# revision 16
# speedup vs baseline: 1.0673x; 1.0673x over previous
"""Trainium2 Bass kernel for the LRU (Linear Recurrent Unit) nn.Module.

Math
----
Reference computes, per timestep t (T=4096, H=2048, N=1024):
    Bu_t   = B_norm @ u_t                    (complex, B_norm = (B_re+iB_im)*gamma)
    h_t    = lambda * h_{t-1} + Bu_t         (diagonal complex recurrence)
    y_t    = Re(C @ h_t) + D * u_t

Device strategy (8 NeuronCores, tensor-parallel over d_hidden N):
Each core owns NSH = N/8 = 128 channels.  With lambda_n = r_n * exp(i*theta_n)
the rotating-frame substitution g_t = exp(-i*theta*t) * h_t turns the complex
recurrence into two *real* scans
    g_t = r * g_{t-1} + exp(-i*theta*t) * Bu_t
which map 1:1 onto the VectorE tensor_tensor_scan instruction.  Rotation
tables cos(theta_n*t), sin(theta_n*t) are precomputed on host in float64.

Per core:
  mm1  (TensorE):  Bu.T = BnT.T @ x.T        -> [NSH, T] (re,im) in PSUM
  rot-in (VectorE): w = exp(-i theta t) Bu   -> SBUF
  scan (VectorE):  g = scan(r, w)            (chunked, carried via `initial`)
  rot-out (VectorE): h = exp(+i theta t) g   -> f32r SBUF
  mm2  (TensorE):  y_part = h_re.T @ C_re.T - h_im.T @ C_im.T  -> [T, H]
Host gathers: y = sum_c y_part_c + D * u.

The emission is software-pipelined: mm2 of chunk c is emitted after mm1 of
chunk c+1 so the TensorE never waits on the VectorE scan chain (keeps the
PE HAM clock warm).
"""

import os

import numpy as np

T, H, N = 4096, 2048, 1024
NCORES = 8
NSH = N // NCORES  # 128 channels per core
TCH = 512          # time chunk (= max fp32 matmul moving free dim = 1 PSUM bank)
NCHUNK = T // TCH  # 8
KT = H // 128      # 16 contraction tiles in mm1
HCH = 512          # h chunk in mm2
NHC = H // HCH     # 4

_CACHE = {}

# last BassKernelResults (for test harness introspection)
last_results = None


def _build_program():
    import concourse.mybir as mybir
    from concourse import bacc
    from concourse.tile import TileContext

    F32 = mybir.dt.float32
    BF16 = mybir.dt.bfloat16
    MUL = mybir.AluOpType.mult
    ADD = mybir.AluOpType.add
    SUB = mybir.AluOpType.subtract

    nc = bacc.Bacc("TRN2", target_bir_lowering=False, debug=False,
                   num_devices=NCORES)

    xT = nc.dram_tensor("xT", [128, NCHUNK * KT * TCH], BF16,
                        kind="ExternalInput").ap()
    bn_re = nc.dram_tensor("bn_re", [128, KT * NSH], BF16,
                           kind="ExternalInput").ap()
    bn_im = nc.dram_tensor("bn_im", [128, KT * NSH], BF16,
                           kind="ExternalInput").ap()
    ct_re = nc.dram_tensor("ct_re", [NSH, H], BF16, kind="ExternalInput").ap()
    ct_in = nc.dram_tensor("ct_in", [NSH, H], BF16, kind="ExternalInput").ap()
    # merged per-chunk rotation table: [128, c, (cos|sin), TCH]
    csT = nc.dram_tensor("csT", [NSH, NCHUNK * 2 * TCH], BF16,
                         kind="ExternalInput").ap()
    rvec = nc.dram_tensor("rvec", [NSH, 1], F32, kind="ExternalInput").ap()
    ypart = nc.dram_tensor("ypart", [T, H], BF16, kind="ExternalOutput").ap()

    with TileContext(nc) as tc:
        with (
            tc.tile_pool(name="persist", bufs=1) as pp,
            tc.tile_pool(name="xin", bufs=2) as xp,
            tc.tile_pool(name="rot", bufs=2) as rp,
            tc.tile_pool(name="wbuf", bufs=3) as wp,
            tc.tile_pool(name="gbuf", bufs=3) as gp,
            tc.tile_pool(name="hbuf", bufs=3) as hp,
            tc.tile_pool(name="yout", bufs=3) as yp,
            tc.tile_pool(name="csn", bufs=3) as cp,
            tc.tile_pool(name="ps1", bufs=2, space="PSUM") as ps1,
            tc.tile_pool(name="ps2", bufs=4, space="PSUM") as ps2,
        ):
            # ---- persistent loads ----
            # Critical path: the first matmuls need bn slices + the first x
            # pieces.  Issue those in fine grains split across the two HWDGE
            # queues (sync + scalar) so the PE can start ~2us in.
            bre = pp.tile([128, KT * NSH], BF16, tag="bre")
            bim = pp.tile([128, KT * NSH], BF16, tag="bim")
            nc.scalar.dma_start(bre[:, 0:2 * NSH], bn_re[:, 0:2 * NSH])
            nc.scalar.dma_start(bim[:, 0:2 * NSH], bn_im[:, 0:2 * NSH])
            nc.scalar.dma_start(bre[:, 2 * NSH:], bn_re[:, 2 * NSH:])
            nc.scalar.dma_start(bim[:, 2 * NSH:], bn_im[:, 2 * NSH:])
            rv = pp.tile([128, 1], F32, tag="rv")
            nc.sync.dma_start(rv[:], rvec)
            ctr = pp.tile([128, H], BF16, tag="ctr")
            nc.gpsimd.dma_start(ctr[:], ct_re)
            cti = pp.tile([128, H], BF16, tag="cti")
            nc.gpsimd.dma_start(cti[:], ct_in)
            rbc = pp.tile([128, TCH], F32, tag="rbc")
            nc.vector.tensor_copy(rbc[:], rv[:, 0:1].broadcast_to([128, TCH]))

            prev_gre = prev_gim = None
            hist = []  # pending (chunk, hre, him) awaiting mm2

            def emit_front(c):
                """mm1 + rotations + scans for chunk c."""
                nonlocal prev_gre, prev_gim
                t0 = c * TCH
                # chunk 0 lands in eight fine pieces alternating between the
                # two HWDGE queues (PE can start on piece 0); later chunks
                # arrive as two halves on the sync queue, prefetched a full
                # chunk ahead.
                xt = xp.tile([128, KT * TCH], BF16, tag="xt")
                x0 = c * KT * TCH
                npc = 8 if c == 0 else 2
                PW = KT * TCH // npc
                for q in range(npc):
                    eng = nc.sync if (c > 0 or q % 2 == 0) else nc.scalar
                    eng.dma_start(
                        xt[:, q * PW:(q + 1) * PW],
                        xT[:, x0 + q * PW:x0 + (q + 1) * PW],
                    )
                pre = ps1.tile([128, TCH], F32, tag="pre")
                pim = ps1.tile([128, TCH], F32, tag="pim")
                for a in range(KT):
                    xsl = xt[:, a * TCH:(a + 1) * TCH]
                    nc.tensor.matmul(
                        pre[:], bre[:, a * NSH:(a + 1) * NSH], xsl,
                        start=(a == 0), stop=(a == KT - 1),
                    )
                    nc.tensor.matmul(
                        pim[:], bim[:, a * NSH:(a + 1) * NSH], xsl,
                        start=(a == 0), stop=(a == KT - 1),
                    )
                cs_t = cp.tile([128, 2 * TCH], BF16, tag="csl")
                nc.gpsimd.dma_start(
                    cs_t[:], csT[:, c * 2 * TCH:(c + 1) * 2 * TCH])
                csl = cs_t[:, 0:TCH]
                snl = cs_t[:, TCH:2 * TCH]
                # rotate into the r-frame: w = e^{-i theta t} * Bu
                t1 = rp.tile([128, TCH], F32, tag="t1")
                t2 = rp.tile([128, TCH], F32, tag="t2")
                wre = wp.tile([128, TCH], F32, tag="wre")
                wim = wp.tile([128, TCH], F32, tag="wim")
                nc.vector.tensor_tensor(t1[:], csl, pre[:], op=MUL)
                nc.vector.tensor_tensor(t2[:], snl, pim[:], op=MUL)
                nc.vector.tensor_tensor(wre[:], t1[:], t2[:], op=ADD)
                nc.vector.tensor_tensor(t1[:], csl, pim[:], op=MUL)
                nc.vector.tensor_tensor(t2[:], snl, pre[:], op=MUL)
                nc.vector.tensor_tensor(wim[:], t1[:], t2[:], op=SUB)
                # the two real scans
                gre = gp.tile([128, TCH], F32, tag="gre")
                gim = gp.tile([128, TCH], F32, tag="gim")
                init_re = 0.0 if c == 0 else prev_gre[:, TCH - 1:TCH]
                init_im = 0.0 if c == 0 else prev_gim[:, TCH - 1:TCH]
                nc.vector.tensor_tensor_scan(
                    gre[:], rbc[:], wre[:], init_re, MUL, ADD)
                nc.vector.tensor_tensor_scan(
                    gim[:], rbc[:], wim[:], init_im, MUL, ADD)
                prev_gre, prev_gim = gre, gim
                # rotate back: h = e^{+i theta t} * g
                hre = hp.tile([128, TCH], BF16, tag="hre")
                him = hp.tile([128, TCH], BF16, tag="him")
                nc.vector.tensor_tensor(t1[:], csl, gre[:], op=MUL)
                nc.vector.tensor_tensor(t2[:], snl, gim[:], op=MUL)
                nc.vector.tensor_tensor(hre[:], t1[:], t2[:], op=SUB)
                nc.vector.tensor_tensor(t1[:], csl, gim[:], op=MUL)
                nc.vector.tensor_tensor(t2[:], snl, gre[:], op=MUL)
                nc.vector.tensor_tensor(him[:], t1[:], t2[:], op=ADD)
                hist.append((c, hre, him))

            def emit_back():
                """mm2 + output for the oldest pending chunk."""
                c, hre, him = hist.pop(0)
                t0 = c * TCH
                last_c = c == NCHUNK - 1
                for tt in range(TCH // 128):
                    lre = hre[:, tt * 128:(tt + 1) * 128]
                    lim = him[:, tt * 128:(tt + 1) * 128]
                    last_tt = last_c and tt == TCH // 128 - 1
                    yo = yp.tile([128, H], BF16, tag="yo")
                    pos = []
                    for _ in range(NHC):
                        po = ps2.tile([128, HCH], F32, tag="po")
                        pos.append(po)
                    for hc in range(NHC):
                        nc.tensor.matmul(
                            pos[hc][:], lre, ctr[:, hc * HCH:(hc + 1) * HCH],
                            start=True, stop=False,
                        )
                    # close each bank then copy it out immediately (only the
                    # Scalar engine has PSUM access + spare cycles); ypart
                    # stores go on the sync queue so Scalar does copies only.
                    for hc in range(NHC):
                        nc.tensor.matmul(
                            pos[hc][:], lim, cti[:, hc * HCH:(hc + 1) * HCH],
                            start=False, stop=True,
                        )
                        nc.scalar.copy(yo[:, hc * HCH:(hc + 1) * HCH],
                                       pos[hc][:])
                        if last_tt:
                            nc.gpsimd.dma_start(
                                ypart[t0 + tt * 128:t0 + (tt + 1) * 128,
                                      hc * HCH:(hc + 1) * HCH],
                                yo[:, hc * HCH:(hc + 1) * HCH])
                    if not last_tt:
                        nc.gpsimd.dma_start(
                            ypart[t0 + tt * 128:t0 + (tt + 1) * 128, :],
                            yo[:])

            for c in range(NCHUNK):
                emit_front(c)
                if c >= 1:
                    emit_back()
            emit_back()

    nc.compile()
    return nc


def _arrange_bn(bn_slice):
    import ml_dtypes
    # bn_slice [NSH, H] (float64) -> [128, KT*NSH] with
    # out[p, a*NSH + n] = bn_slice[n, a*128 + p]
    bnT = bn_slice.T.astype(ml_dtypes.bfloat16)  # [H, NSH]
    return np.ascontiguousarray(
        bnT.reshape(KT, 128, NSH).transpose(1, 0, 2)).reshape(128, -1)


def _host_prep(inputs, nu, theta, gamma_log, B_re, B_im, C_re, C_im, D):
    """Float64 host-side precompute; returns per-core input maps."""
    import ml_dtypes
    BF = ml_dtypes.bfloat16
    x = np.asarray(inputs, dtype=np.float32)
    th64 = np.exp(np.asarray(theta).astype(np.float64))
    r64 = np.exp(-np.exp(np.asarray(nu).astype(np.float64)))
    gamma = np.exp(np.asarray(gamma_log).astype(np.float64))
    Bn_re = np.asarray(B_re).astype(np.float64) * gamma[:, None]
    Bn_im = np.asarray(B_im).astype(np.float64) * gamma[:, None]
    t_idx = np.arange(T, dtype=np.float64)
    phase = th64[:, None] * t_idx[None, :]
    cos_all = np.cos(phase).astype(BF)  # [N, T]
    sin_all = np.sin(phase).astype(BF)
    # merged per-chunk layout: cs_all[n, c, 0|1, t] = cos|sin(theta_n*(c*TCH+t))
    cs_all = np.stack(
        [cos_all.reshape(N, NCHUNK, TCH), sin_all.reshape(N, NCHUNK, TCH)],
        axis=2).reshape(N, NCHUNK * 2 * TCH)
    # pre-arrange x into the per-chunk SBUF layout:
    # xTa[p, c, a, t] = x[c*TCH + t, a*128 + p]
    xTa = np.ascontiguousarray(
        x.reshape(NCHUNK, TCH, KT, 128).transpose(3, 0, 2, 1).astype(BF)
    ).reshape(128, -1)
    C_re = np.asarray(C_re, dtype=np.float32).astype(BF)
    C_im = np.asarray(C_im, dtype=np.float32).astype(BF)

    in_maps = []
    for c in range(NCORES):
        sl = slice(c * NSH, (c + 1) * NSH)
        in_maps.append({
            "xT": xTa,
            "bn_re": _arrange_bn(Bn_re[sl]),
            "bn_im": _arrange_bn(Bn_im[sl]),
            "ct_re": np.ascontiguousarray(C_re[:, sl].T),
            "ct_in": np.ascontiguousarray(-C_im[:, sl].T),
            "csT": np.ascontiguousarray(cs_all[sl]),
            "rvec": np.ascontiguousarray(r64[sl].astype(np.float32)[:, None]),
        })
    return in_maps


def kernel(inputs, nu, theta, gamma_log, B_re, B_im, C_re, C_im, D):
    global last_results
    from concourse.bass_utils import run_bass_kernel_spmd

    if "nc" not in _CACHE:
        _CACHE["nc"] = _build_program()
    nc = _CACHE["nc"]

    in_maps = _host_prep(
        inputs, nu, theta, gamma_log, B_re, B_im, C_re, C_im, D)

    trace = os.environ.get("LRU_TRACE") == "1"
    res = run_bass_kernel_spmd(
        nc, in_maps, core_ids=list(range(NCORES)), trace=trace)
    last_results = res

    y64 = np.zeros((T, H), np.float64)
    for r in res.results:
        y64 += r["ypart"].astype(np.float64)
    y64 += (np.asarray(D).astype(np.float64)[None, :]
            * np.asarray(inputs).astype(np.float64))
    return y64.astype(np.float32)



# revision 20
# speedup vs baseline: 1.0917x; 1.0228x over previous
"""Trainium2 Bass kernel for the LRU (Linear Recurrent Unit) nn.Module.

Math
----
Reference computes, per timestep t (T=4096, H=2048, N=1024):
    Bu_t   = B_norm @ u_t                    (complex, B_norm = (B_re+iB_im)*gamma)
    h_t    = lambda * h_{t-1} + Bu_t         (diagonal complex recurrence)
    y_t    = Re(C @ h_t) + D * u_t

Device strategy (8 NeuronCores, tensor-parallel over d_hidden N):
Each core owns NSH = N/8 = 128 channels.  With lambda_n = r_n * exp(i*theta_n)
the rotating-frame substitution g_t = exp(-i*theta*t) * h_t turns the complex
recurrence into two *real* scans
    g_t = r * g_{t-1} + exp(-i*theta*t) * Bu_t
which map 1:1 onto the VectorE tensor_tensor_scan instruction.  Rotation
tables cos(theta_n*t), sin(theta_n*t) are precomputed on host in float64.

Per core:
  mm1  (TensorE):  Bu.T = BnT.T @ x.T        -> [NSH, T] (re,im) in PSUM
  rot-in (VectorE): w = exp(-i theta t) Bu   -> SBUF
  scan (VectorE):  g = scan(r, w)            (chunked, carried via `initial`)
  rot-out (VectorE): h = exp(+i theta t) g   -> f32r SBUF
  mm2  (TensorE):  y_part = h_re.T @ C_re.T - h_im.T @ C_im.T  -> [T, H]
Host gathers: y = sum_c y_part_c + D * u.

The emission is software-pipelined: mm2 of chunk c is emitted after mm1 of
chunk c+1 so the TensorE never waits on the VectorE scan chain (keeps the
PE HAM clock warm).
"""

import os

import numpy as np

T, H, N = 4096, 2048, 1024
NCORES = 8
NSH = N // NCORES  # 128 channels per core
TCH = 512          # time chunk (= max fp32 matmul moving free dim = 1 PSUM bank)
NCHUNK = T // TCH  # 8
KT = H // 128      # 16 contraction tiles in mm1
HCH = 512          # h chunk in mm2
NHC = H // HCH     # 4

_CACHE = {}

# last BassKernelResults (for test harness introspection)
last_results = None


def _build_program():
    import concourse.mybir as mybir
    from concourse import bacc
    from concourse.tile import TileContext

    F32 = mybir.dt.float32
    BF16 = mybir.dt.bfloat16
    MUL = mybir.AluOpType.mult
    ADD = mybir.AluOpType.add
    SUB = mybir.AluOpType.subtract

    nc = bacc.Bacc("TRN2", target_bir_lowering=False, debug=False,
                   num_devices=NCORES)

    xT = nc.dram_tensor("xT", [128, NCHUNK * KT * TCH], BF16,
                        kind="ExternalInput").ap()
    bn_re = nc.dram_tensor("bn_re", [128, KT * NSH], BF16,
                           kind="ExternalInput").ap()
    bn_im = nc.dram_tensor("bn_im", [128, KT * NSH], BF16,
                           kind="ExternalInput").ap()
    ct_re = nc.dram_tensor("ct_re", [NSH, H], BF16, kind="ExternalInput").ap()
    ct_in = nc.dram_tensor("ct_in", [NSH, H], BF16, kind="ExternalInput").ap()
    # merged per-chunk rotation table: [128, c, (cos|sin), TCH]
    csT = nc.dram_tensor("csT", [NSH, NCHUNK * 2 * TCH], BF16,
                         kind="ExternalInput").ap()
    rvec = nc.dram_tensor("rvec", [NSH, 1], F32, kind="ExternalInput").ap()
    ypart = nc.dram_tensor("ypart", [T, H], BF16, kind="ExternalOutput").ap()

    with TileContext(nc) as tc:
        with (
            tc.tile_pool(name="persist", bufs=1) as pp,
            tc.tile_pool(name="xin", bufs=3) as xp,
            tc.tile_pool(name="rot", bufs=2) as rp,
            tc.tile_pool(name="wbuf", bufs=3) as wp,
            tc.tile_pool(name="gbuf", bufs=3) as gp,
            tc.tile_pool(name="hbuf", bufs=3) as hp,
            tc.tile_pool(name="yout", bufs=3) as yp,
            tc.tile_pool(name="csn", bufs=3) as cp,
            tc.tile_pool(name="ps1", bufs=2, space="PSUM") as ps1,
            tc.tile_pool(name="ps2", bufs=4, space="PSUM") as ps2,
        ):
            # ---- persistent loads ----
            # bn goes on the otherwise-idle scalar HWDGE queue (first slices
            # first) so it lands in parallel with chunk 0's x quarters on the
            # sync queue.  C waits behind the chunk-0 rotation table on the
            # gpsimd queue (mm2 starts much later than rot-in).
            bre = pp.tile([128, KT * NSH], BF16, tag="bre")
            bim = pp.tile([128, KT * NSH], BF16, tag="bim")
            nc.scalar.dma_start(bre[:, 0:2 * NSH], bn_re[:, 0:2 * NSH])
            nc.scalar.dma_start(bim[:, 0:2 * NSH], bn_im[:, 0:2 * NSH])
            nc.scalar.dma_start(bre[:, 2 * NSH:], bn_re[:, 2 * NSH:])
            nc.scalar.dma_start(bim[:, 2 * NSH:], bn_im[:, 2 * NSH:])
            rv = pp.tile([128, 1], F32, tag="rv")
            ctr = pp.tile([128, H], BF16, tag="ctr")
            cti = pp.tile([128, H], BF16, tag="cti")
            rbc = pp.tile([128, TCH], F32, tag="rbc")

            def emit_persist_rest():
                nc.sync.dma_start(rv[:], rvec)
                nc.gpsimd.dma_start(ctr[:], ct_re)
                nc.gpsimd.dma_start(cti[:], ct_in)
                nc.vector.tensor_copy(
                    rbc[:], rv[:, 0:1].broadcast_to([128, TCH]))

            prev_gre = prev_gim = None
            hist = []  # pending (chunk, hre, him) awaiting mm2

            def emit_front(c):
                """mm1 + rotations + scans for chunk c."""
                nonlocal prev_gre, prev_gim
                t0 = c * TCH
                # four quarter-DMAs on the sync queue so the first matmuls
                # start before the whole chunk has landed
                xt = xp.tile([128, KT * TCH], BF16, tag="xt")
                x0 = c * KT * TCH
                QW = KT * TCH // 4
                for q in range(4):
                    nc.sync.dma_start(
                        xt[:, q * QW:(q + 1) * QW],
                        xT[:, x0 + q * QW:x0 + (q + 1) * QW],
                    )
                cs_t = cp.tile([128, 2 * TCH], BF16, tag="csl")
                nc.gpsimd.dma_start(
                    cs_t[:], csT[:, c * 2 * TCH:(c + 1) * 2 * TCH])
                csl = cs_t[:, 0:TCH]
                snl = cs_t[:, TCH:2 * TCH]
                if c == 0:
                    emit_persist_rest()
                pre = ps1.tile([128, TCH], F32, tag="pre")
                pim = ps1.tile([128, TCH], F32, tag="pim")
                for a in range(KT):
                    xsl = xt[:, a * TCH:(a + 1) * TCH]
                    nc.tensor.matmul(
                        pre[:], bre[:, a * NSH:(a + 1) * NSH], xsl,
                        start=(a == 0), stop=(a == KT - 1),
                    )
                    nc.tensor.matmul(
                        pim[:], bim[:, a * NSH:(a + 1) * NSH], xsl,
                        start=(a == 0), stop=(a == KT - 1),
                    )
                # rotate into the r-frame: w = e^{-i theta t} * Bu
                t1 = rp.tile([128, TCH], F32, tag="t1")
                t2 = rp.tile([128, TCH], F32, tag="t2")
                wre = wp.tile([128, TCH], F32, tag="wre")
                wim = wp.tile([128, TCH], F32, tag="wim")
                nc.vector.tensor_tensor(t1[:], csl, pre[:], op=MUL)
                nc.vector.tensor_tensor(t2[:], snl, pim[:], op=MUL)
                nc.vector.tensor_tensor(wre[:], t1[:], t2[:], op=ADD)
                nc.vector.tensor_tensor(t1[:], csl, pim[:], op=MUL)
                nc.vector.tensor_tensor(t2[:], snl, pre[:], op=MUL)
                nc.vector.tensor_tensor(wim[:], t1[:], t2[:], op=SUB)
                # the two real scans
                gre = gp.tile([128, TCH], F32, tag="gre")
                gim = gp.tile([128, TCH], F32, tag="gim")
                init_re = 0.0 if c == 0 else prev_gre[:, TCH - 1:TCH]
                init_im = 0.0 if c == 0 else prev_gim[:, TCH - 1:TCH]
                nc.vector.tensor_tensor_scan(
                    gre[:], rbc[:], wre[:], init_re, MUL, ADD)
                nc.vector.tensor_tensor_scan(
                    gim[:], rbc[:], wim[:], init_im, MUL, ADD)
                prev_gre, prev_gim = gre, gim
                # rotate back: h = e^{+i theta t} * g
                hre = hp.tile([128, TCH], BF16, tag="hre")
                him = hp.tile([128, TCH], BF16, tag="him")
                nc.vector.tensor_tensor(t1[:], csl, gre[:], op=MUL)
                nc.vector.tensor_tensor(t2[:], snl, gim[:], op=MUL)
                nc.vector.tensor_tensor(hre[:], t1[:], t2[:], op=SUB)
                nc.vector.tensor_tensor(t1[:], csl, gim[:], op=MUL)
                nc.vector.tensor_tensor(t2[:], snl, gre[:], op=MUL)
                nc.vector.tensor_tensor(him[:], t1[:], t2[:], op=ADD)
                hist.append((c, hre, him))

            def emit_back():
                """mm2 + output for the oldest pending chunk."""
                c, hre, him = hist.pop(0)
                t0 = c * TCH
                last_c = c == NCHUNK - 1
                for tt in range(TCH // 128):
                    lre = hre[:, tt * 128:(tt + 1) * 128]
                    lim = him[:, tt * 128:(tt + 1) * 128]
                    last_tt = last_c and tt == TCH // 128 - 1
                    yo = yp.tile([128, H], BF16, tag="yo")
                    pos = []
                    for _ in range(NHC):
                        po = ps2.tile([128, HCH], F32, tag="po")
                        pos.append(po)
                    for hc in range(NHC):
                        nc.tensor.matmul(
                            pos[hc][:], lre, ctr[:, hc * HCH:(hc + 1) * HCH],
                            start=True, stop=False,
                        )
                    # close each bank then copy it out immediately (only the
                    # Scalar engine has PSUM access + spare cycles).  Bulk
                    # ypart stores ride the gpsimd queue; the last chunk's
                    # stores switch to the two HWDGE queues, which are idle
                    # by then and flush much faster (shorter kernel tail).
                    for hc in range(NHC):
                        nc.tensor.matmul(
                            pos[hc][:], lim, cti[:, hc * HCH:(hc + 1) * HCH],
                            start=False, stop=True,
                        )
                        nc.scalar.copy(yo[:, hc * HCH:(hc + 1) * HCH],
                                       pos[hc][:])
                        if last_c:
                            eng = nc.sync if hc % 2 == 0 else nc.scalar
                            eng.dma_start(
                                ypart[t0 + tt * 128:t0 + (tt + 1) * 128,
                                      hc * HCH:(hc + 1) * HCH],
                                yo[:, hc * HCH:(hc + 1) * HCH])
                    if not last_c:
                        nc.gpsimd.dma_start(
                            ypart[t0 + tt * 128:t0 + (tt + 1) * 128, :],
                            yo[:])

            for c in range(NCHUNK):
                emit_front(c)
                if c >= 1:
                    emit_back()
            emit_back()

    nc.compile()
    return nc


def _arrange_bn(bn_slice):
    import ml_dtypes
    # bn_slice [NSH, H] (float64) -> [128, KT*NSH] with
    # out[p, a*NSH + n] = bn_slice[n, a*128 + p]
    bnT = bn_slice.T.astype(ml_dtypes.bfloat16)  # [H, NSH]
    return np.ascontiguousarray(
        bnT.reshape(KT, 128, NSH).transpose(1, 0, 2)).reshape(128, -1)


def _host_prep(inputs, nu, theta, gamma_log, B_re, B_im, C_re, C_im, D):
    """Float64 host-side precompute; returns per-core input maps."""
    import ml_dtypes
    BF = ml_dtypes.bfloat16
    x = np.asarray(inputs, dtype=np.float32)
    th64 = np.exp(np.asarray(theta).astype(np.float64))
    r64 = np.exp(-np.exp(np.asarray(nu).astype(np.float64)))
    gamma = np.exp(np.asarray(gamma_log).astype(np.float64))
    Bn_re = np.asarray(B_re).astype(np.float64) * gamma[:, None]
    Bn_im = np.asarray(B_im).astype(np.float64) * gamma[:, None]
    t_idx = np.arange(T, dtype=np.float64)
    phase = th64[:, None] * t_idx[None, :]
    cos_all = np.cos(phase).astype(BF)  # [N, T]
    sin_all = np.sin(phase).astype(BF)
    # merged per-chunk layout: cs_all[n, c, 0|1, t] = cos|sin(theta_n*(c*TCH+t))
    cs_all = np.stack(
        [cos_all.reshape(N, NCHUNK, TCH), sin_all.reshape(N, NCHUNK, TCH)],
        axis=2).reshape(N, NCHUNK * 2 * TCH)
    # pre-arrange x into the per-chunk SBUF layout:
    # xTa[p, c, a, t] = x[c*TCH + t, a*128 + p]
    xTa = np.ascontiguousarray(
        x.reshape(NCHUNK, TCH, KT, 128).transpose(3, 0, 2, 1).astype(BF)
    ).reshape(128, -1)
    C_re = np.asarray(C_re, dtype=np.float32).astype(BF)
    C_im = np.asarray(C_im, dtype=np.float32).astype(BF)

    in_maps = []
    for c in range(NCORES):
        sl = slice(c * NSH, (c + 1) * NSH)
        in_maps.append({
            "xT": xTa,
            "bn_re": _arrange_bn(Bn_re[sl]),
            "bn_im": _arrange_bn(Bn_im[sl]),
            "ct_re": np.ascontiguousarray(C_re[:, sl].T),
            "ct_in": np.ascontiguousarray(-C_im[:, sl].T),
            "csT": np.ascontiguousarray(cs_all[sl]),
            "rvec": np.ascontiguousarray(r64[sl].astype(np.float32)[:, None]),
        })
    return in_maps


def kernel(inputs, nu, theta, gamma_log, B_re, B_im, C_re, C_im, D):
    global last_results
    from concourse.bass_utils import run_bass_kernel_spmd

    if "nc" not in _CACHE:
        _CACHE["nc"] = _build_program()
    nc = _CACHE["nc"]

    in_maps = _host_prep(
        inputs, nu, theta, gamma_log, B_re, B_im, C_re, C_im, D)

    trace = os.environ.get("LRU_TRACE") == "1"
    res = run_bass_kernel_spmd(
        nc, in_maps, core_ids=list(range(NCORES)), trace=trace)
    last_results = res

    y64 = np.zeros((T, H), np.float64)
    for r in res.results:
        y64 += r["ypart"].astype(np.float64)
    y64 += (np.asarray(D).astype(np.float64)[None, :]
            * np.asarray(inputs).astype(np.float64))
    return y64.astype(np.float32)



# revision 21
# speedup vs baseline: 1.0973x; 1.0052x over previous
"""Trainium2 Bass kernel for the LRU (Linear Recurrent Unit) nn.Module.

Math
----
Reference computes, per timestep t (T=4096, H=2048, N=1024):
    Bu_t   = B_norm @ u_t                    (complex, B_norm = (B_re+iB_im)*gamma)
    h_t    = lambda * h_{t-1} + Bu_t         (diagonal complex recurrence)
    y_t    = Re(C @ h_t) + D * u_t

Device strategy (8 NeuronCores, tensor-parallel over d_hidden N):
Each core owns NSH = N/8 = 128 channels.  With lambda_n = r_n * exp(i*theta_n)
the rotating-frame substitution g_t = exp(-i*theta*t) * h_t turns the complex
recurrence into two *real* scans
    g_t = r * g_{t-1} + exp(-i*theta*t) * Bu_t
which map 1:1 onto the VectorE tensor_tensor_scan instruction.  Rotation
tables cos(theta_n*t), sin(theta_n*t) are precomputed on host in float64.

Per core (all matmul operands bf16, accumulation + scan in f32):
  mm1  (TensorE):  Bu.T = BnT.T @ x.T        -> [NSH, T] (re,im) in PSUM
  rot-in (VectorE): w = exp(-i theta t) Bu   -> SBUF
  scan (VectorE):  g = scan(r, w)            (chunked, carried via `initial`)
  rot-out (VectorE): h = exp(+i theta t) g   -> bf16 SBUF
  mm2  (TensorE):  y_part = h_re.T @ C_re.T - h_im.T @ C_im.T  -> [T, H]
Host gathers: y = sum_c y_part_c + D * u  (float64).

The emission is software-pipelined: mm2 of chunk c is emitted after mm1 of
chunk c+1 so the TensorE never waits on the VectorE scan chain.  The last
chunk's output stores ride the two HWDGE queues (idle by then) instead of
the gpsimd software-DGE queue, shortening the kernel tail.
"""

import os

import numpy as np

T, H, N = 4096, 2048, 1024
NCORES = 8
NSH = N // NCORES  # 128 channels per core
TCH = 512          # time chunk (= max matmul moving free dim = 1 PSUM bank)
NCHUNK = T // TCH  # 8
KT = H // 128      # 16 contraction tiles in mm1
HCH = 512          # h chunk in mm2
NHC = H // HCH     # 4

_CACHE = {}

# last BassKernelResults (for test harness introspection)
last_results = None


def _build_program():
    import concourse.mybir as mybir
    from concourse import bacc
    from concourse.tile import TileContext

    F32 = mybir.dt.float32
    BF16 = mybir.dt.bfloat16
    MUL = mybir.AluOpType.mult
    ADD = mybir.AluOpType.add
    SUB = mybir.AluOpType.subtract

    nc = bacc.Bacc("TRN2", target_bir_lowering=False, debug=False,
                   num_devices=NCORES)

    xT = nc.dram_tensor("xT", [128, NCHUNK * KT * TCH], BF16,
                        kind="ExternalInput").ap()
    bn_re = nc.dram_tensor("bn_re", [128, KT * NSH], BF16,
                           kind="ExternalInput").ap()
    bn_im = nc.dram_tensor("bn_im", [128, KT * NSH], BF16,
                           kind="ExternalInput").ap()
    ct_re = nc.dram_tensor("ct_re", [NSH, H], BF16, kind="ExternalInput").ap()
    ct_in = nc.dram_tensor("ct_in", [NSH, H], BF16, kind="ExternalInput").ap()
    cosT = nc.dram_tensor("cosT", [NSH, T], BF16, kind="ExternalInput").ap()
    sinT = nc.dram_tensor("sinT", [NSH, T], BF16, kind="ExternalInput").ap()
    rvec = nc.dram_tensor("rvec", [NSH, 1], F32, kind="ExternalInput").ap()
    ypart = nc.dram_tensor("ypart", [T, H], BF16, kind="ExternalOutput").ap()

    with TileContext(nc) as tc:
        with (
            tc.tile_pool(name="persist", bufs=1) as pp,
            tc.tile_pool(name="xin", bufs=2) as xp,
            tc.tile_pool(name="rot", bufs=2) as rp,
            tc.tile_pool(name="wbuf", bufs=3) as wp,
            tc.tile_pool(name="gbuf", bufs=3) as gp,
            tc.tile_pool(name="hbuf", bufs=3) as hp,
            tc.tile_pool(name="yout", bufs=3) as yp,
            tc.tile_pool(name="csn", bufs=3) as cp,
            tc.tile_pool(name="ps1", bufs=2, space="PSUM") as ps1,
            tc.tile_pool(name="ps2", bufs=4, space="PSUM") as ps2,
        ):
            # ---- persistent loads ----
            # bn_re/bn_im feed the first matmuls: keep them on the sync queue,
            # everything bulky that is needed later goes to the gpsimd queue.
            bre = pp.tile([128, KT * NSH], BF16, tag="bre")
            nc.sync.dma_start(bre[:], bn_re)
            bim = pp.tile([128, KT * NSH], BF16, tag="bim")
            nc.sync.dma_start(bim[:], bn_im)
            rv = pp.tile([128, 1], F32, tag="rv")
            nc.sync.dma_start(rv[:], rvec)
            ctr = pp.tile([128, H], BF16, tag="ctr")
            nc.gpsimd.dma_start(ctr[:], ct_re)
            cti = pp.tile([128, H], BF16, tag="cti")
            nc.gpsimd.dma_start(cti[:], ct_in)
            rbc = pp.tile([128, TCH], F32, tag="rbc")
            nc.vector.tensor_copy(rbc[:], rv[:, 0:1].broadcast_to([128, TCH]))

            prev_gre = prev_gim = None
            hist = []  # pending (chunk, hre, him) awaiting mm2

            def emit_front(c):
                """mm1 + rotations + scans for chunk c."""
                nonlocal prev_gre, prev_gim
                t0 = c * TCH
                # four quarter-DMAs from the host-pre-arranged layout so the
                # first matmuls can start before the whole chunk has landed
                xt = xp.tile([128, KT * TCH], BF16, tag="xt")
                x0 = c * KT * TCH
                QW = KT * TCH // 4
                for q in range(4):
                    nc.sync.dma_start(
                        xt[:, q * QW:(q + 1) * QW],
                        xT[:, x0 + q * QW:x0 + (q + 1) * QW],
                    )
                pre = ps1.tile([128, TCH], F32, tag="pre")
                pim = ps1.tile([128, TCH], F32, tag="pim")
                for a in range(KT):
                    xsl = xt[:, a * TCH:(a + 1) * TCH]
                    nc.tensor.matmul(
                        pre[:], bre[:, a * NSH:(a + 1) * NSH], xsl,
                        start=(a == 0), stop=(a == KT - 1),
                    )
                    nc.tensor.matmul(
                        pim[:], bim[:, a * NSH:(a + 1) * NSH], xsl,
                        start=(a == 0), stop=(a == KT - 1),
                    )
                csl_t = cp.tile([128, TCH], BF16, tag="csl")
                snl_t = cp.tile([128, TCH], BF16, tag="snl")
                nc.gpsimd.dma_start(csl_t[:], cosT[:, t0:t0 + TCH])
                nc.gpsimd.dma_start(snl_t[:], sinT[:, t0:t0 + TCH])
                csl = csl_t[:]
                snl = snl_t[:]
                # rotate into the r-frame: w = e^{-i theta t} * Bu
                t1 = rp.tile([128, TCH], F32, tag="t1")
                t2 = rp.tile([128, TCH], F32, tag="t2")
                wre = wp.tile([128, TCH], F32, tag="wre")
                wim = wp.tile([128, TCH], F32, tag="wim")
                nc.vector.tensor_tensor(t1[:], csl, pre[:], op=MUL)
                nc.vector.tensor_tensor(t2[:], snl, pim[:], op=MUL)
                nc.vector.tensor_tensor(wre[:], t1[:], t2[:], op=ADD)
                nc.vector.tensor_tensor(t1[:], csl, pim[:], op=MUL)
                nc.vector.tensor_tensor(t2[:], snl, pre[:], op=MUL)
                nc.vector.tensor_tensor(wim[:], t1[:], t2[:], op=SUB)
                # the two real scans
                gre = gp.tile([128, TCH], F32, tag="gre")
                gim = gp.tile([128, TCH], F32, tag="gim")
                init_re = 0.0 if c == 0 else prev_gre[:, TCH - 1:TCH]
                init_im = 0.0 if c == 0 else prev_gim[:, TCH - 1:TCH]
                nc.vector.tensor_tensor_scan(
                    gre[:], rbc[:], wre[:], init_re, MUL, ADD)
                nc.vector.tensor_tensor_scan(
                    gim[:], rbc[:], wim[:], init_im, MUL, ADD)
                prev_gre, prev_gim = gre, gim
                # rotate back: h = e^{+i theta t} * g
                hre = hp.tile([128, TCH], BF16, tag="hre")
                him = hp.tile([128, TCH], BF16, tag="him")
                nc.vector.tensor_tensor(t1[:], csl, gre[:], op=MUL)
                nc.vector.tensor_tensor(t2[:], snl, gim[:], op=MUL)
                nc.vector.tensor_tensor(hre[:], t1[:], t2[:], op=SUB)
                nc.vector.tensor_tensor(t1[:], csl, gim[:], op=MUL)
                nc.vector.tensor_tensor(t2[:], snl, gre[:], op=MUL)
                nc.vector.tensor_tensor(him[:], t1[:], t2[:], op=ADD)
                hist.append((c, hre, him))

            def emit_back():
                """mm2 + output for the oldest pending chunk."""
                c, hre, him = hist.pop(0)
                t0 = c * TCH
                last_c = c == NCHUNK - 1
                for tt in range(TCH // 128):
                    lre = hre[:, tt * 128:(tt + 1) * 128]
                    lim = him[:, tt * 128:(tt + 1) * 128]
                    yo = yp.tile([128, H], BF16, tag="yo")
                    pos = []
                    for _ in range(NHC):
                        po = ps2.tile([128, HCH], F32, tag="po")
                        pos.append(po)
                    for hc in range(NHC):
                        nc.tensor.matmul(
                            pos[hc][:], lre, ctr[:, hc * HCH:(hc + 1) * HCH],
                            start=True, stop=False,
                        )
                    for hc in range(NHC):
                        nc.tensor.matmul(
                            pos[hc][:], lim, cti[:, hc * HCH:(hc + 1) * HCH],
                            start=False, stop=True,
                        )
                    for hc in range(NHC):
                        nc.scalar.copy(yo[:, hc * HCH:(hc + 1) * HCH],
                                       pos[hc][:])
                    if last_c:
                        # tail: the HWDGE queues are idle and flush faster
                        # than the gpsimd software DGE
                        for hc in range(NHC):
                            eng = nc.sync if hc % 2 == 0 else nc.scalar
                            eng.dma_start(
                                ypart[t0 + tt * 128:t0 + (tt + 1) * 128,
                                      hc * HCH:(hc + 1) * HCH],
                                yo[:, hc * HCH:(hc + 1) * HCH])
                    else:
                        nc.gpsimd.dma_start(
                            ypart[t0 + tt * 128:t0 + (tt + 1) * 128, :],
                            yo[:])

            for c in range(NCHUNK):
                emit_front(c)
                if c >= 1:
                    emit_back()
            emit_back()

    nc.compile()
    return nc


def _arrange_bn(bn_slice):
    import ml_dtypes
    # bn_slice [NSH, H] (float64) -> [128, KT*NSH] with
    # out[p, a*NSH + n] = bn_slice[n, a*128 + p]
    bnT = bn_slice.T.astype(ml_dtypes.bfloat16)  # [H, NSH]
    return np.ascontiguousarray(
        bnT.reshape(KT, 128, NSH).transpose(1, 0, 2)).reshape(128, -1)


def _host_prep(inputs, nu, theta, gamma_log, B_re, B_im, C_re, C_im, D):
    """Float64 host-side precompute; returns per-core input maps."""
    import ml_dtypes
    BF = ml_dtypes.bfloat16
    x = np.asarray(inputs, dtype=np.float32)
    th64 = np.exp(np.asarray(theta).astype(np.float64))
    r64 = np.exp(-np.exp(np.asarray(nu).astype(np.float64)))
    gamma = np.exp(np.asarray(gamma_log).astype(np.float64))
    Bn_re = np.asarray(B_re).astype(np.float64) * gamma[:, None]
    Bn_im = np.asarray(B_im).astype(np.float64) * gamma[:, None]
    t_idx = np.arange(T, dtype=np.float64)
    phase = th64[:, None] * t_idx[None, :]
    cos_all = np.cos(phase).astype(BF)  # [N, T]
    sin_all = np.sin(phase).astype(BF)
    # pre-arrange x into the per-chunk SBUF layout:
    # xTa[p, c, a, t] = x[c*TCH + t, a*128 + p]
    xTa = np.ascontiguousarray(
        x.reshape(NCHUNK, TCH, KT, 128).transpose(3, 0, 2, 1).astype(BF)
    ).reshape(128, -1)
    C_re = np.asarray(C_re, dtype=np.float32).astype(BF)
    C_im = np.asarray(C_im, dtype=np.float32).astype(BF)

    in_maps = []
    for c in range(NCORES):
        sl = slice(c * NSH, (c + 1) * NSH)
        in_maps.append({
            "xT": xTa,
            "bn_re": _arrange_bn(Bn_re[sl]),
            "bn_im": _arrange_bn(Bn_im[sl]),
            "ct_re": np.ascontiguousarray(C_re[:, sl].T),
            "ct_in": np.ascontiguousarray(-C_im[:, sl].T),
            "cosT": np.ascontiguousarray(cos_all[sl]),
            "sinT": np.ascontiguousarray(sin_all[sl]),
            "rvec": np.ascontiguousarray(r64[sl].astype(np.float32)[:, None]),
        })
    return in_maps


def kernel(inputs, nu, theta, gamma_log, B_re, B_im, C_re, C_im, D):
    global last_results
    from concourse.bass_utils import run_bass_kernel_spmd

    if "nc" not in _CACHE:
        _CACHE["nc"] = _build_program()
    nc = _CACHE["nc"]

    in_maps = _host_prep(
        inputs, nu, theta, gamma_log, B_re, B_im, C_re, C_im, D)

    trace = os.environ.get("LRU_TRACE") == "1"
    res = run_bass_kernel_spmd(
        nc, in_maps, core_ids=list(range(NCORES)), trace=trace)
    last_results = res

    y64 = np.zeros((T, H), np.float64)
    for r in res.results:
        y64 += r["ypart"].astype(np.float64)
    y64 += (np.asarray(D).astype(np.float64)[None, :]
            * np.asarray(inputs).astype(np.float64))
    return y64.astype(np.float32)
